# revision 26
# baseline (speedup 1.0000x reference)
"""Causal single-head attention (B=4, S=4096, D=1024, H=128) on 8 trn2 cores.

Sharding: 2 cores per batch.  Core parity p takes every other 128-row
q-block (global q-block = 2v+p).  KV columns are fed to each core in a
parity-permuted order (adjacent 128-blocks swapped for p=1) so that every
core's q-blocks sit at even *virtual* positions — all 8 cores then run one
identical SPMD program with perfectly balanced causal work:
virtual q-chunk j (512 rows) attends virtual kv-chunks 0..2j+1, the last
two of which carry a data-supplied 0/1 mask.

Per-core dataflow (projections fp32r = full PE rate; attention value path
bf16 to halve DVE work and SBUF traffic):
  xT tiles ->  KT[h,kv] / V[kv,h] / QT[h,q] projections (biases folded:
               bq,bk via ACT bias; bv,bo folded into a host-side bias)
  scoresT[kv,q] = KT_blk^T @ QT_chunk   (PSUM, fp32r)
  ex = ACT Exp(scale*s) PSUM->SBUF bf16; diagonal chunks 0/1-masked on DVE
  outT[h,q]  += V_blk^T @ ex            (PSUM accumulate over kv, bf16 mm)
  exs = ex0 + ex1 on DVE (pair-sum);  denom += ones^T @ exs — one PE
        matmul per kv-block *pair* instead of per block
  out = (outT * 1/denom)^T @ WoT        -> DMA out
Softmax max-subtraction is skipped: logits are ~N(0,0.17) so exp is safe.

Schedule: projection chains for slice s2+1 and the output projection of
q-chunk j-1 are chopped into units and round-robined between attention
items of q-chunk j, so the ACT exp stream (which ~matches PE attention
throughput) always has PE projection work to hide behind.

DMA: x is loaded 1024-col slice at a time; slice 0 fine-grained across all
four issuing queues (SP/ACT HWDGE + Pool/DVE SWDGE) to cut the startup
stall, later slices as two [P,4,1024] transfers.  Outputs of q-chunks 0-2
leave as single [P,4,1024] transfers; the final chunk is split per-block
across queues to shorten the kernel tail.
"""

import sys

sys.path.insert(0, "/opt/trn_rl_repo")

import numpy as np

import concourse.bass as bass
import concourse.tile as tile
from concourse import mybir
from concourse.vector_clock import ScopedClock

P = 128
D = 1024
S = 4096
B = 4
H = 128
NCORES = 8
SCALE = 1.0 / float(np.sqrt(H))

F32 = mybir.dt.float32
F32R = mybir.dt.float32r
BF16 = mybir.dt.bfloat16

_patched = [False]


def _patch_tile_drain():
    """The walrus build in this container rejects instructions with more
    than one sync-wait command; spread the Tile kernel-tail drain's
    global-clock waits over single-wait nops."""
    if _patched[0]:
        return
    _patched[0] = True

    def _drain_and_barrier(self, tick_clock, wait_clock):
        nc = self.nc
        probe = nc.sync.nop(nofuse=True)
        wait_clock.add_sem_waits(
            probe.ins, ScopedClock({None: tick_clock.global_clock})
        )
        si = probe.ins.sync_info
        waits = list(si.on_wait) if (si and si.on_wait) else []
        if len(waits) > 1:
            si.on_wait = waits[:1]
            for w in waits[1:]:
                n = nc.sync.nop(nofuse=True)
                nsi = n.ins.sync_info
                if nsi is None:
                    n.ins.sync_info = mybir.SyncInfo(on_wait=[w], on_update=[])
                else:
                    nsi.on_wait = [w]
        nc.sync.drain()
        nc.all_engine_barrier()
        popped = nc._tile_sem_poison_stack.pop()
        assert popped is self._sem_poison
        nc.clear_and_free_semaphores(list(self.sems.allocated().values()))
        nc.all_engine_barrier()

    tile.TileContext._drain_and_barrier = _drain_and_barrier


def _split_excess_waits(nc, max_waits=1):
    """Hoist all but max_waits sync-waits from each instruction onto
    same-engine nops placed immediately before it."""
    for fn in nc.m.functions:
        for bb in fn.blocks:
            new_insts = []
            for inst in bb.instructions:
                si = inst.sync_info
                if si is not None and si.on_wait and len(si.on_wait) > max_waits:
                    waits = list(si.on_wait)
                    for w in waits[:-max_waits]:
                        nop = mybir.InstNoOp(
                            name=nc.get_next_instruction_name(),
                            sync_info=mybir.SyncInfo(on_wait=[w], on_update=[]),
                            bass_nofuse=True,
                            engine=inst.engine,
                        )
                        nc.register_instruction(nop)
                        new_insts.append(nop)
                    si.on_wait = waits[-max_waits:]
                new_insts.append(inst)
            bb.instructions[:] = new_insts


def build_program(d=D, s=S, variant="base"):
    """One uniform per-core program; differences between cores live in data.

    variant: timing-probe builds ("base" is the real kernel):
      nopv    — attention runs scores+exp but only 1 PV/ones matmul per chunk
      noattn  — projections + output DMA only
      dmaonly — input/output DMA only, no compute
    """
    _patch_tile_drain()
    from contextlib import ExitStack

    DC = d // P            # contraction chunks (8)
    NKVB = s // P          # kv 128-blocks (32)
    NSC = s // 512         # kv 512-chunks (8)
    SQ = s // 2            # queries per core (2048)
    NQC = SQ // 512        # q-chunks (4)

    nc = bass.Bass("TRN2", target_bir_lowering=False, debug=False,
                   num_devices=NCORES)

    xT = nc.declare_dram_parameter("xT", [d, s], BF16, isOutput=False)
    wq = nc.declare_dram_parameter("wq", [P, (d // P) * H], BF16,
                                   isOutput=False)
    wk = nc.declare_dram_parameter("wk", [P, (d // P) * H], BF16,
                                   isOutput=False)
    wv = nc.declare_dram_parameter("wv", [P, (d // P) * H], BF16,
                                   isOutput=False)
    wo = nc.declare_dram_parameter("wo", [H, d], F32R, isOutput=False)
    # packed small constants: bqk = [bq | bk], aux = [ident | ones | mask]
    bqkd = nc.declare_dram_parameter("bqk", [H, 2], F32, isOutput=False)
    auxd = nc.declare_dram_parameter("aux", [P, P + 8 * 512], BF16,
                                     isOutput=False)
    outd = nc.declare_dram_parameter("out", [SQ, d], F32, isOutput=True)

    xTr = xT.rearrange("(c p) s -> p c s", p=P)
    outr = outd.rearrange("(b p) d -> p b d", p=P)

    with tile.TileContext(nc) as tc, ExitStack() as ctx:
        singles = ctx.enter_context(tc.tile_pool(name="singles", bufs=1))
        xt_pool = ctx.enter_context(tc.tile_pool(name="xt", bufs=2))
        exp_pool = ctx.enter_context(tc.tile_pool(name="expp", bufs=6))
        exs_pool = ctx.enter_context(tc.tile_pool(name="exsp", bufs=4))
        misc = ctx.enter_context(tc.tile_pool(name="misc", bufs=4))
        fin_pool = ctx.enter_context(tc.tile_pool(name="fin", bufs=2))
        ps_a = ctx.enter_context(tc.tile_pool(name="psa", bufs=2, space="PSUM"))
        ps_acc = ctx.enter_context(tc.tile_pool(name="psacc", bufs=2, space="PSUM"))
        ps_s = ctx.enter_context(tc.tile_pool(name="pss", bufs=2, space="PSUM"))

        wk_s = singles.tile([P, DC, H], BF16)
        wq_s = singles.tile([P, DC, H], BF16)
        wv_s = singles.tile([P, DC, H], BF16)
        bqk_s = singles.tile([P, 2], F32)
        bq_s = bqk_s[:, 0:1]
        bk_s = bqk_s[:, 1:2]

        # resident projection outputs
        KT = singles.tile([P, s], F32R)        # [h, kv]
        Vn = singles.tile([P, NKVB, P], BF16)  # [kv%128, kvblock, h]
        QT = singles.tile([P, SQ], F32R)       # [h, q]
        otn_all = singles.tile([P, NQC, 512], F32R)  # normalized outT per j

        wo_s = singles.tile([P, d], F32R)
        aux_s = singles.tile([P, P + 8 * 512], BF16)
        ones_s = aux_s[:, 0:P]
        mask_s = aux_s[:, P:].rearrange("p (e c) -> p e c", c=512)

        def emit_x_dma(s2):
            """Issue the input transfers for one 1024-col slice of x."""
            xt = xt_pool.tile([P, DC, 1024], BF16, tag="xt")
            lo, hi = s2 * 1024, (s2 + 1) * 1024
            if s2 == 0:
                # startup: 1MB granules across the three issuing queues, with
                # the early-needed weights ordered first
                nc.gpsimd.dma_start(
                    out=wk_s[:], in_=wk.rearrange("p (c h) -> p c h", h=H))
                nc.sync.dma_start(out=xt[:, 0:2, :], in_=xTr[:, 0:2, lo:hi])
                nc.sync.dma_start(out=bqk_s[:], in_=bqkd[:])
                nc.scalar.dma_start(out=xt[:, 2:4, :], in_=xTr[:, 2:4, lo:hi])
                nc.gpsimd.dma_start(
                    out=wv_s[:], in_=wv.rearrange("p (c h) -> p c h", h=H))
                nc.sync.dma_start(out=xt[:, 4:6, :], in_=xTr[:, 4:6, lo:hi])
                nc.scalar.dma_start(out=xt[:, 6:8, :], in_=xTr[:, 6:8, lo:hi])
                nc.scalar.dma_start(
                    out=wq_s[:], in_=wq.rearrange("p (c h) -> p c h", h=H))
                nc.gpsimd.dma_start(out=aux_s[:], in_=auxd[:])
                nc.gpsimd.dma_start(out=wo_s[:], in_=wo[:])
            else:
                nc.sync.dma_start(out=xt[:, :4, :], in_=xTr[:, 0:4, lo:hi])
                nc.gpsimd.dma_start(out=xt[:, 4:, :], in_=xTr[:, 4:, lo:hi])
            return xt

        def make_proj_units(s2, xt):
            """PE work for one x slice, chopped into 5 schedulable units."""
            units = []
            for c in range(2):
                sc = 2 * s2 + c
                off = c * 512

                def kunit(sc=sc, off=off):
                    kt_ps = ps_a.tile([P, 512], F32, tag="b512")
                    for dc in range(DC):
                        nc.tensor.matmul(out=kt_ps[:], lhsT=wk_s[:, dc, :],
                                         rhs=xt[:, dc, off:off + 512],
                                         start=(dc == 0), stop=(dc == DC - 1))
                    nc.scalar.activation(
                        out=KT[:, sc * 512:(sc + 1) * 512], in_=kt_ps[:],
                        func=mybir.ActivationFunctionType.Identity,
                        bias=bk_s[:],
                    )

                def vunit(sc=sc, off=off):
                    # V computed directly in [kv, h] layout: per kv-block,
                    # lhsT = the x 128-col block (stationary), rhs = wv chunk
                    vps = ps_a.tile([P, 512], F32, tag="b512")
                    for blk in range(4):
                        for dc in range(DC):
                            nc.tensor.matmul(
                                out=vps[:, blk * P:(blk + 1) * P],
                                lhsT=xt[:, dc, off + blk * P:
                                        off + (blk + 1) * P],
                                rhs=wv_s[:, dc, :],
                                start=(dc == 0), stop=(dc == DC - 1),
                            )
                    nc.vector.tensor_copy(
                        out=Vn[:, sc * 4:(sc + 1) * 4, :],
                        in_=vps.rearrange("p (b c) -> p b c", c=P),
                    )

                units += [kunit, vunit]

            def qunit():
                q_ps = ps_a.tile([P, 512], F32, tag="b512")
                for dc in range(DC):
                    rhs8 = xt[:, dc, :].rearrange("p (b c) -> p b c", c=P)
                    nc.tensor.matmul(
                        out=q_ps.rearrange("p (b c) -> p b c", c=P),
                        lhsT=wq_s[:, dc, :],
                        rhs=rhs8[:, ::2, :],
                        start=(dc == 0), stop=(dc == DC - 1),
                    )
                nc.scalar.activation(
                    out=QT[:, s2 * 512:(s2 + 1) * 512], in_=q_ps[:],
                    func=mybir.ActivationFunctionType.Identity, bias=bq_s[:],
                )

            units.append(qunit)
            return units

        def outproj_block(jj, blk, fin, tail=False):
            for half in range(d // 512):
                fo_ps = ps_a.tile([P, 512], F32, tag="b512")
                nc.tensor.matmul(
                    out=fo_ps[:],
                    lhsT=otn_all[:, jj, blk * P:(blk + 1) * P],
                    rhs=wo_s[:, half * 512:(half + 1) * 512],
                    start=True, stop=True,
                )
                ceng = (nc.vector.tensor_copy
                        if (blk + half) % 2 == 0 else nc.scalar.copy)
                ceng(out=fin[:, blk, half * 512:(half + 1) * 512],
                     in_=fo_ps[:])
            if tail:
                oeng = nc.sync if blk % 2 == 0 else nc.gpsimd
                oeng.dma_start(
                    out=outd[(4 * jj + blk) * P:(4 * jj + blk + 1) * P, :],
                    in_=fin[:, blk, :],
                )
            elif blk == 1:
                nc.sync.dma_start(
                    out=outr[:, 4 * jj:4 * jj + 2, :], in_=fin[:, 0:2, :])
            elif blk == 3:
                nc.gpsimd.dma_start(
                    out=outr[:, 4 * jj + 2:4 * jj + 4, :], in_=fin[:, 2:4, :])

        def make_outproj_units(jj, tail=False):
            """Output projection of q-chunk jj as 4 per-block units."""
            fin = fin_pool.tile([P, 4, d], F32, tag="fin")
            return [
                (lambda blk=blk: outproj_block(jj, blk, fin, tail))
                for blk in range(4)
            ]

        # ---- attention body for one q-chunk; `units` (projection work for
        # slice j+1 + output projection of chunk j-1) round-robined in ----
        def emit_attention(j, units):
            nkv = 2 * j + 2           # kv 512-chunks attended
            npairs = 2 * nkv          # score tiles of 2 kv-blocks each
            qs = j * 512

            acc_ps = ps_acc.tile([P, 512], F32, tag="acc")  # outT accumulator
            den_ps = ps_acc.tile([P, 512], F32, tag="acc")  # denominator rows

            # Masked (diagonal) blocks first: their extra DVE mask latency
            # then overlaps the remaining unmasked blocks' PE work instead of
            # stalling the tail of the accumulation chain.
            plain = [(m, 0, 512, None) for m in range(npairs - 4)]
            maskA = [(npairs - 4 + i, 0, 512, 2 * i) for i in range(2)]
            maskB = [(npairs - 2 + i, 256, 256, 4 + 2 * i) for i in range(2)]
            tail = j == NQC - 1
            if tail:
                # half-masked pairs last: q-columns 0:256 of acc/den are then
                # final two pv-emissions early, letting normalize + the first
                # half of the output projection overlap the attention tail
                items = plain[:2] + maskA + plain[2:] + maskB
            else:
                items = plain[:2] + maskA + maskB + plain[2:]
            nit = len(items)
            uq = list(units)

            def emit_pv(mi, m, off, w, ex, exs):
                if variant == "nopv":
                    if mi == 0:
                        nc.tensor.matmul(out=den_ps[:], lhsT=ones_s[:],
                                         rhs=exs[:], start=True, stop=True)
                        nc.tensor.matmul(out=acc_ps[:], lhsT=Vn[:, 2 * m, :],
                                         rhs=ex[:, 0, :], start=True,
                                         stop=True)
                    return
                last = mi == nit - 1
                for t in range(2):
                    nc.tensor.matmul(
                        out=acc_ps[:, off:off + w],
                        lhsT=Vn[:, 2 * m + t, :], rhs=ex[:, t, :w],
                        start=(mi == 0 and t == 0), stop=(last and t == 1),
                    )
                nc.tensor.matmul(
                    out=den_ps[:, off:off + w], lhsT=ones_s[:],
                    rhs=exs[:, :w], start=(mi == 0), stop=last,
                )

            # Software pipeline, depth 2: pair m's PV/ones are emitted after
            # pair m+2's score matmuls, so exp + mask latency never stalls PE.
            pending = []
            for mi, (m, off, w, e0) in enumerate(items):
                sc_ps = ps_s.tile([P, 2, 512], F32)
                for t in range(2):
                    kvb = 2 * m + t
                    nc.tensor.matmul(
                        out=sc_ps[:, t, :w],
                        lhsT=KT[:, kvb * P:(kvb + 1) * P],
                        rhs=QT[:, qs + off:qs + off + w],
                        start=True, stop=True,
                    )
                ex = exp_pool.tile([P, 2, 512], BF16)
                nc.scalar.activation(
                    out=ex[:, :, :w], in_=sc_ps[:, :, :w],
                    func=mybir.ActivationFunctionType.Exp, scale=SCALE,
                )
                if e0 is not None:
                    nc.vector.tensor_mul(
                        out=ex[:, :, :w], in0=ex[:, :, :w],
                        in1=mask_s[:, e0:e0 + 2, off:off + w],
                    )
                exs = exs_pool.tile([P, 512], BF16)
                nc.vector.tensor_add(out=exs[:, :w], in0=ex[:, 0, :w],
                                     in1=ex[:, 1, :w])
                pending.append((mi, m, off, w, ex, exs))
                if len(pending) > 2:
                    emit_pv(*pending.pop(0))
                # keep interleaved units on pace: after item mi, the number
                # of units left should be at most nu*(nit-1-mi)/nit
                while uq and len(uq) * nit > len(units) * (nit - 1 - mi):
                    uq.pop(0)()
            recip = misc.tile([P, 512], F32, tag="recip")
            if tail and variant == "base":
                while uq:
                    uq.pop(0)()
                for args in pending:
                    emit_pv(*args)
                # kernel tail: per-128-col normalize so each output-projection
                # block launches as soon as its columns are ready
                fin = fin_pool.tile([P, 4, d], F32, tag="fin")
                for blk in range(4):
                    lo = blk * P
                    nc.vector.reciprocal(out=recip[:, lo:lo + P],
                                         in_=den_ps[:, lo:lo + P])
                    nc.vector.tensor_mul(out=otn_all[:, j, lo:lo + P],
                                         in0=acc_ps[:, lo:lo + P],
                                         in1=recip[:, lo:lo + P])
                    outproj_block(j, blk, fin, tail=True)
                return
            for args in pending:
                emit_pv(*args)
            while uq:
                uq.pop(0)()
            nc.vector.reciprocal(out=recip[:], in_=den_ps[:])
            nc.vector.tensor_mul(out=otn_all[:, j, :], in0=acc_ps[:],
                                 in1=recip[:])

        # ---- top-level schedule ----
        if variant == "dmaonly":
            for s2 in range(NSC // 2):
                xt = emit_x_dma(s2)
                for blk in range(4):
                    vq = 4 * s2 + blk
                    fin = fin_pool.tile([P, d], F32, tag="dfin")
                    nc.vector.tensor_copy(out=fin[:], in_=xt[:, blk, :])
                    oeng = nc.sync if blk % 2 == 0 else nc.gpsimd
                    oeng.dma_start(out=outd[vq * P:(vq + 1) * P, :],
                                   in_=fin[:])
        elif variant == "noattn":
            for s2 in range(NSC // 2):
                xt = emit_x_dma(s2)
                for u in make_proj_units(s2, xt):
                    u()
                nc.vector.tensor_copy(out=otn_all[:, s2, :],
                                      in_=QT[:, s2 * 512:(s2 + 1) * 512])
                for u in make_outproj_units(s2, tail=(s2 == NSC // 2 - 1)):
                    u()
        else:
            xt = emit_x_dma(0)
            for u in make_proj_units(0, xt):
                u()
            for j in range(NQC):
                units = []
                if j > 0:
                    units += make_outproj_units(j - 1)
                if j + 1 < NQC:
                    xt = emit_x_dma(j + 1)
                    units += make_proj_units(j + 1, xt)
                emit_attention(j, units)
            if variant != "base":
                for u in make_outproj_units(NQC - 1, tail=True):
                    u()

    _split_excess_waits(nc)
    return nc


def make_masks(p, dtype=np.float32):
    """mask[e, t, 128u+r] = 1 iff virtual-kv (block e, offset t) is attended
    by virtual-q (block u, offset r) of the same 512-aligned q-chunk."""
    e = np.arange(8)[:, None, None]
    t = np.arange(P)[None, :, None]
    sq = np.arange(512)[None, None, :]
    u, r = sq // P, sq % P
    kv_pos = (e ^ p) * P + t
    q_pos = 256 * u + P * p + r
    return (kv_pos <= q_pos).astype(dtype)


def shard_inputs(x, Wq, bq, Wk, bk, Wv, bv, Wo, bo):
    """Build per-core input maps (and the host-side residual bias)."""
    import ml_dtypes

    bf16 = ml_dtypes.bfloat16
    x = np.asarray(x, dtype=np.float32)

    def packw(W):
        # [D, H] -> [P, DC*H] with row dc*128+p at [p, dc*H:(dc+1)*H]:
        # contiguous 2KB per partition line for descriptor-friendly DMA
        wt = np.asarray(W, np.float32).T.astype(bf16)          # [D, H]
        return np.ascontiguousarray(
            wt.reshape(D // P, P, H).transpose(1, 0, 2).reshape(P, -1))

    wq_t, wk_t, wv_t = packw(Wq), packw(Wk), packw(Wv)
    wo_t = np.ascontiguousarray(np.asarray(Wo, np.float32).T)  # [H, D]
    bqk = np.stack([np.asarray(bq, np.float32),
                    np.asarray(bk, np.float32)], axis=1)  # [H, 2]
    # aux = [ones | mask] packed along the free dim, bf16
    auxes = []
    for p in range(2):
        aux = np.empty((P, P + 8 * 512), bf16)
        aux[:, 0:P] = 1
        aux[:, P:] = np.ascontiguousarray(
            make_masks(p, bf16).transpose(1, 0, 2)).reshape(P, 8 * 512)
        auxes.append(aux)
    # bv and bo are applied on the host: softmax rows sum to one, so
    # attn@(V+bv) @ Wo^T + bo = attn@V @ Wo^T + (Wo@bv + bo).
    bo_eff = (np.asarray(Wo, np.float32) @ np.asarray(bv, np.float32)
              + np.asarray(bo, np.float32))

    in_maps = []
    nblk = S // P
    for c in range(NCORES):
        b, p = c // 2, c % 2
        xb = x[b]
        if p:
            perm = np.arange(nblk) ^ 1
            xb = xb.reshape(nblk, P, D)[perm].reshape(S, D)
        xT = np.ascontiguousarray(xb.T.astype(bf16))
        in_maps.append({
            "xT": xT, "wq": wq_t, "wk": wk_t, "wv": wv_t, "wo": wo_t,
            "bqk": bqk, "aux": auxes[p],
        })
    return in_maps, bo_eff


def gather_outputs(results, bo_eff):
    out = np.empty((B, S, D), np.float32)
    for c in range(NCORES):
        b, p = c // 2, c % 2
        co = results[c]["out"]           # [S//2, D]
        blocks = co.reshape(S // 2 // P, P, D)
        out[b, :, :].reshape(S // P, P, D)[2 * np.arange(S // 2 // P) + p] = blocks
    out += bo_eff[None, None, :]
    return out


_prog_cache = {}


def _get_program():
    if "nc" not in _prog_cache:
        _prog_cache["nc"] = build_program()
    return _prog_cache["nc"]


def kernel(x, Wq, bq, Wk, bk, Wv, bv, Wo, bo):
    from concourse.bass_utils import run_bass_kernel_spmd

    nc = _get_program()
    in_maps, bo_eff = shard_inputs(x, Wq, bq, Wk, bk, Wv, bv, Wo, bo)
    res = run_bass_kernel_spmd(nc, in_maps, core_ids=list(range(NCORES)))
    return gather_outputs(res.results, bo_eff)


# revision 31
# speedup vs baseline: 16.2459x; 16.2459x over previous
"""Causal single-head attention (B=4, S=4096, D=1024, H=128) on 8 trn2 cores.

Sharding: 2 cores per batch.  Core parity p takes every other 128-row
q-block (global q-block = 2v+p).  KV columns are fed to each core in a
parity-permuted order (adjacent 128-blocks swapped for p=1) so that every
core's q-blocks sit at even *virtual* positions — all 8 cores then run one
identical SPMD program with perfectly balanced causal work:
virtual q-chunk j (512 rows) attends virtual kv-chunks 0..2j+1, the last
two of which carry a data-supplied 0/1 mask.

Per-core dataflow (projections fp32r = full PE rate; attention value path
bf16 to halve DVE work and SBUF traffic):
  xT tiles ->  KT[h,kv] / V[kv,h] / QT[h,q] projections (biases folded:
               bq,bk via ACT bias; bv,bo folded into a host-side bias)
  scoresT[kv,q] = KT_blk^T @ QT_chunk   (PSUM, fp32r)
  ex = ACT Exp(scale*s) PSUM->SBUF bf16; diagonal chunks 0/1-masked on DVE
  outT[h,q]  += V_blk^T @ ex            (PSUM accumulate over kv, bf16 mm)
  exs = ex0 + ex1 on DVE (pair-sum);  denom += ones^T @ exs — one PE
        matmul per kv-block *pair* instead of per block
  out = (outT * 1/denom)^T @ WoT        -> DMA out
Softmax max-subtraction is skipped: logits are ~N(0,0.17) so exp is safe.

Schedule: projection chains for slice s2+1 and the output projection of
q-chunk j-1 are chopped into units and round-robined between attention
items of q-chunk j, so the ACT exp stream (which ~matches PE attention
throughput) always has PE projection work to hide behind.

DMA: x is loaded 1024-col slice at a time; slice 0 fine-grained across all
four issuing queues (SP/ACT HWDGE + Pool/DVE SWDGE) to cut the startup
stall, later slices as two [P,4,1024] transfers.  Outputs of q-chunks 0-2
leave as single [P,4,1024] transfers; the final chunk is split per-block
across queues to shorten the kernel tail.
"""

import sys

sys.path.insert(0, "/opt/trn_rl_repo")

import numpy as np

import concourse.bass as bass
import concourse.tile as tile
from concourse import mybir
from concourse.vector_clock import ScopedClock

P = 128
D = 1024
S = 4096
B = 4
H = 128
NCORES = 8
SCALE = 1.0 / float(np.sqrt(H))

F32 = mybir.dt.float32
F32R = mybir.dt.float32r
BF16 = mybir.dt.bfloat16

_patched = [False]


def _patch_tile_drain():
    """The walrus build in this container rejects instructions with more
    than one sync-wait command; spread the Tile kernel-tail drain's
    global-clock waits over single-wait nops."""
    if _patched[0]:
        return
    _patched[0] = True

    def _drain_and_barrier(self, tick_clock, wait_clock):
        nc = self.nc
        probe = nc.sync.nop(nofuse=True)
        wait_clock.add_sem_waits(
            probe.ins, ScopedClock({None: tick_clock.global_clock})
        )
        si = probe.ins.sync_info
        waits = list(si.on_wait) if (si and si.on_wait) else []
        if len(waits) > 1:
            si.on_wait = waits[:1]
            for w in waits[1:]:
                n = nc.sync.nop(nofuse=True)
                nsi = n.ins.sync_info
                if nsi is None:
                    n.ins.sync_info = mybir.SyncInfo(on_wait=[w], on_update=[])
                else:
                    nsi.on_wait = [w]
        nc.sync.drain()
        nc.all_engine_barrier()
        popped = nc._tile_sem_poison_stack.pop()
        assert popped is self._sem_poison
        nc.clear_and_free_semaphores(list(self.sems.allocated().values()))
        nc.all_engine_barrier()

    tile.TileContext._drain_and_barrier = _drain_and_barrier


def _split_excess_waits(nc, max_waits=1):
    """Hoist all but max_waits sync-waits from each instruction onto
    same-engine nops placed immediately before it."""
    for fn in nc.m.functions:
        for bb in fn.blocks:
            new_insts = []
            for inst in bb.instructions:
                si = inst.sync_info
                if si is not None and si.on_wait and len(si.on_wait) > max_waits:
                    waits = list(si.on_wait)
                    for w in waits[:-max_waits]:
                        nop = mybir.InstNoOp(
                            name=nc.get_next_instruction_name(),
                            sync_info=mybir.SyncInfo(on_wait=[w], on_update=[]),
                            bass_nofuse=True,
                            engine=inst.engine,
                        )
                        nc.register_instruction(nop)
                        new_insts.append(nop)
                    si.on_wait = waits[-max_waits:]
                new_insts.append(inst)
            bb.instructions[:] = new_insts


def build_program(d=D, s=S, variant="base"):
    """One uniform per-core program; differences between cores live in data.

    variant: timing-probe builds ("base" is the real kernel):
      nopv    — attention runs scores+exp but only 1 PV/ones matmul per chunk
      noattn  — projections + output DMA only
      dmaonly — input/output DMA only, no compute
    """
    _patch_tile_drain()
    from contextlib import ExitStack

    DC = d // P            # contraction chunks (8)
    NKVB = s // P          # kv 128-blocks (32)
    NSC = s // 512         # kv 512-chunks (8)
    SQ = s // 2            # queries per core (2048)
    NQC = SQ // 512        # q-chunks (4)

    nc = bass.Bass("TRN2", target_bir_lowering=False, debug=False,
                   num_devices=NCORES)

    CW = 3 * (d // P) * H + d + 2 + P + 8 * 512  # packed const width
    xT = nc.declare_dram_parameter("xT", [d, s], BF16, isOutput=False)
    constd = nc.declare_dram_parameter("const", [P, CW], BF16, isOutput=False)
    outd = nc.declare_dram_parameter("out", [SQ, d], F32, isOutput=True)
    # const layout (free-dim offsets)
    O_WK, O_BQK, O_WV, O_WQ = 0, d, d + 2, 2 * d + 2
    O_WO, O_ONES, O_MASK = 3 * d + 2, 4 * d + 2, 4 * d + 2 + P

    xTr = xT.rearrange("(c p) s -> p c s", p=P)
    outr = outd.rearrange("(b p) d -> p b d", p=P)

    with tile.TileContext(nc) as tc, ExitStack() as ctx:
        singles = ctx.enter_context(tc.tile_pool(name="singles", bufs=1))
        xt_pool = ctx.enter_context(tc.tile_pool(name="xt", bufs=2))
        exp_pool = ctx.enter_context(tc.tile_pool(name="expp", bufs=6))
        exs_pool = ctx.enter_context(tc.tile_pool(name="exsp", bufs=4))
        misc = ctx.enter_context(tc.tile_pool(name="misc", bufs=4))
        fin_pool = ctx.enter_context(tc.tile_pool(name="fin", bufs=2))
        ps_a = ctx.enter_context(tc.tile_pool(name="psa", bufs=2, space="PSUM"))
        ps_acc = ctx.enter_context(tc.tile_pool(name="psacc", bufs=2, space="PSUM"))
        ps_s = ctx.enter_context(tc.tile_pool(name="pss", bufs=2, space="PSUM"))

        const_s = singles.tile([P, CW], BF16)

        def wview(off):
            return const_s[:, off:off + d].rearrange("p (c h) -> p c h", h=H)

        wk_s, wq_s, wv_s = wview(O_WK), wview(O_WQ), wview(O_WV)
        wo_s = const_s[:, O_WO:O_WO + d]
        bqk_s = singles.tile([P, 2], F32)
        bq_s = bqk_s[:, 0:1]
        bk_s = bqk_s[:, 1:2]

        # resident projection outputs
        KT = singles.tile([P, s], F32R)        # [h, kv]
        Vn = singles.tile([P, NKVB, P], BF16)  # [kv%128, kvblock, h]
        QT = singles.tile([P, SQ], F32R)       # [h, q]
        otn_all = singles.tile([P, NQC, 512], BF16)  # normalized outT per j

        ones_s = const_s[:, O_ONES:O_ONES + P]
        mask_s = const_s[:, O_MASK:].rearrange("p (e c) -> p e c", c=512)

        def emit_x_dma(s2):
            """Issue the input transfers for one 1024-col slice of x."""
            xt = xt_pool.tile([P, DC, 1024], BF16, tag="xt")
            lo, hi = s2 * 1024, (s2 + 1) * 1024
            if s2 == 0:
                # startup: 1MB granules across the three issuing queues, with
                # the early-needed weights ordered first
                nc.gpsimd.dma_start(out=const_s[:, O_WK:O_WK + d + 2],
                                    in_=constd[:, O_WK:O_WK + d + 2])
                nc.scalar.copy(out=bqk_s[:],
                               in_=const_s[:, O_BQK:O_BQK + 2])
                nc.sync.dma_start(out=xt[:, 0:2, :], in_=xTr[:, 0:2, lo:hi])
                nc.scalar.dma_start(out=xt[:, 2:4, :], in_=xTr[:, 2:4, lo:hi])
                nc.gpsimd.dma_start(out=const_s[:, O_WV:O_WV + d],
                                    in_=constd[:, O_WV:O_WV + d])
                nc.sync.dma_start(out=xt[:, 4:6, :], in_=xTr[:, 4:6, lo:hi])
                nc.scalar.dma_start(out=xt[:, 6:8, :], in_=xTr[:, 6:8, lo:hi])
                nc.scalar.dma_start(out=const_s[:, O_WQ:O_WQ + d],
                                    in_=constd[:, O_WQ:O_WQ + d])
                nc.gpsimd.dma_start(out=const_s[:, O_ONES:],
                                    in_=constd[:, O_ONES:])
                nc.gpsimd.dma_start(out=const_s[:, O_WO:O_WO + d],
                                    in_=constd[:, O_WO:O_WO + d])
            else:
                nc.sync.dma_start(out=xt[:, :4, :], in_=xTr[:, 0:4, lo:hi])
                nc.gpsimd.dma_start(out=xt[:, 4:, :], in_=xTr[:, 4:, lo:hi])
            return xt

        def make_proj_units(s2, xt):
            """PE work for one x slice, chopped into 5 schedulable units."""
            units = []
            for c in range(2):
                sc = 2 * s2 + c
                off = c * 512

                def kunit(sc=sc, off=off):
                    kt_ps = ps_a.tile([P, 512], F32, tag="b512")
                    for dc in range(DC):
                        nc.tensor.matmul(out=kt_ps[:], lhsT=wk_s[:, dc, :],
                                         rhs=xt[:, dc, off:off + 512],
                                         start=(dc == 0), stop=(dc == DC - 1))
                    nc.scalar.activation(
                        out=KT[:, sc * 512:(sc + 1) * 512], in_=kt_ps[:],
                        func=mybir.ActivationFunctionType.Identity,
                        bias=bk_s[:],
                    )

                def vunit(sc=sc, off=off):
                    # V computed directly in [kv, h] layout: per kv-block,
                    # lhsT = the x 128-col block (stationary), rhs = wv chunk
                    vps = ps_a.tile([P, 512], F32, tag="b512")
                    for blk in range(4):
                        for dc in range(DC):
                            nc.tensor.matmul(
                                out=vps[:, blk * P:(blk + 1) * P],
                                lhsT=xt[:, dc, off + blk * P:
                                        off + (blk + 1) * P],
                                rhs=wv_s[:, dc, :],
                                start=(dc == 0), stop=(dc == DC - 1),
                            )
                    nc.vector.tensor_copy(
                        out=Vn[:, sc * 4:(sc + 1) * 4, :],
                        in_=vps.rearrange("p (b c) -> p b c", c=P),
                    )

                units += [kunit, vunit]

            def qunit():
                q_ps = ps_a.tile([P, 512], F32, tag="b512")
                for dc in range(DC):
                    rhs8 = xt[:, dc, :].rearrange("p (b c) -> p b c", c=P)
                    nc.tensor.matmul(
                        out=q_ps.rearrange("p (b c) -> p b c", c=P),
                        lhsT=wq_s[:, dc, :],
                        rhs=rhs8[:, ::2, :],
                        start=(dc == 0), stop=(dc == DC - 1),
                    )
                nc.scalar.activation(
                    out=QT[:, s2 * 512:(s2 + 1) * 512], in_=q_ps[:],
                    func=mybir.ActivationFunctionType.Identity, bias=bq_s[:],
                )

            units.append(qunit)
            return units

        def outproj_block(jj, blk, fin, tail=False):
            for half in range(d // 512):
                fo_ps = ps_a.tile([P, 512], F32, tag="b512")
                nc.tensor.matmul(
                    out=fo_ps[:],
                    lhsT=otn_all[:, jj, blk * P:(blk + 1) * P],
                    rhs=wo_s[:, half * 512:(half + 1) * 512],
                    start=True, stop=True,
                )
                ceng = (nc.vector.tensor_copy
                        if (blk + half) % 2 == 0 else nc.scalar.copy)
                ceng(out=fin[:, blk, half * 512:(half + 1) * 512],
                     in_=fo_ps[:])
            if tail:
                oeng = nc.sync if blk % 2 == 0 else nc.gpsimd
                oeng.dma_start(
                    out=outd[(4 * jj + blk) * P:(4 * jj + blk + 1) * P, :],
                    in_=fin[:, blk, :],
                )
            elif blk == 1:
                nc.sync.dma_start(
                    out=outr[:, 4 * jj:4 * jj + 2, :], in_=fin[:, 0:2, :])
            elif blk == 3:
                nc.gpsimd.dma_start(
                    out=outr[:, 4 * jj + 2:4 * jj + 4, :], in_=fin[:, 2:4, :])

        def make_outproj_units(jj, tail=False):
            """Output projection of q-chunk jj as 4 per-block units."""
            fin = fin_pool.tile([P, 4, d], F32, tag="fin")
            return [
                (lambda blk=blk: outproj_block(jj, blk, fin, tail))
                for blk in range(4)
            ]

        # ---- attention body for one q-chunk; `units` (projection work for
        # slice j+1 + output projection of chunk j-1) round-robined in ----
        def emit_attention(j, units):
            nkv = 2 * j + 2           # kv 512-chunks attended
            npairs = 2 * nkv          # score tiles of 2 kv-blocks each
            qs = j * 512

            acc_ps = ps_acc.tile([P, 512], F32, tag="acc")  # outT accumulator
            den_ps = ps_acc.tile([P, 512], F32, tag="acc")  # denominator rows

            # Masked (diagonal) blocks first: their extra DVE mask latency
            # then overlaps the remaining unmasked blocks' PE work instead of
            # stalling the tail of the accumulation chain.
            plain = [(m, 0, 512, None) for m in range(npairs - 4)]
            maskA = [(npairs - 4 + i, 0, 512, 2 * i) for i in range(2)]
            maskB = [(npairs - 2 + i, 256, 256, 4 + 2 * i) for i in range(2)]
            tail = j == NQC - 1
            if tail:
                # half-masked pairs last: q-columns 0:256 of acc/den are then
                # final two pv-emissions early, letting normalize + the first
                # half of the output projection overlap the attention tail
                items = plain[:2] + maskA + plain[2:] + maskB
            else:
                items = plain[:2] + maskA + maskB + plain[2:]
            nit = len(items)
            uq = list(units)

            def emit_pv(mi, m, off, w, ex, exs):
                if variant == "nopv":
                    if mi == 0:
                        nc.tensor.matmul(out=den_ps[:], lhsT=ones_s[:],
                                         rhs=exs[:], start=True, stop=True)
                        nc.tensor.matmul(out=acc_ps[:], lhsT=Vn[:, 2 * m, :],
                                         rhs=ex[:, 0, :], start=True,
                                         stop=True)
                    return
                last = mi == nit - 1
                for t in range(2):
                    nc.tensor.matmul(
                        out=acc_ps[:, off:off + w],
                        lhsT=Vn[:, 2 * m + t, :], rhs=ex[:, t, :w],
                        start=(mi == 0 and t == 0), stop=(last and t == 1),
                    )
                nc.tensor.matmul(
                    out=den_ps[:, off:off + w], lhsT=ones_s[:],
                    rhs=exs[:, :w], start=(mi == 0), stop=last,
                )

            # Software pipeline, depth 2: pair m's PV/ones are emitted after
            # pair m+2's score matmuls, so exp + mask latency never stalls PE.
            pending = []
            for mi, (m, off, w, e0) in enumerate(items):
                sc_ps = ps_s.tile([P, 2, 512], F32)
                for t in range(2):
                    kvb = 2 * m + t
                    nc.tensor.matmul(
                        out=sc_ps[:, t, :w],
                        lhsT=KT[:, kvb * P:(kvb + 1) * P],
                        rhs=QT[:, qs + off:qs + off + w],
                        start=True, stop=True,
                    )
                ex = exp_pool.tile([P, 2, 512], BF16)
                if mi >= nit - 2:
                    # drain-critical items: per-block exp/mask halves the
                    # latency before their PV matmuls can issue
                    for t in range(2):
                        nc.scalar.activation(
                            out=ex[:, t, :w], in_=sc_ps[:, t, :w],
                            func=mybir.ActivationFunctionType.Exp,
                            scale=SCALE,
                        )
                        if e0 is not None:
                            nc.vector.tensor_mul(
                                out=ex[:, t, :w], in0=ex[:, t, :w],
                                in1=mask_s[:, e0 + t, off:off + w],
                            )
                else:
                    nc.scalar.activation(
                        out=ex[:, :, :w], in_=sc_ps[:, :, :w],
                        func=mybir.ActivationFunctionType.Exp, scale=SCALE,
                    )
                    if e0 is not None:
                        nc.vector.tensor_mul(
                            out=ex[:, :, :w], in0=ex[:, :, :w],
                            in1=mask_s[:, e0:e0 + 2, off:off + w],
                        )
                exs = exs_pool.tile([P, 512], BF16)
                nc.vector.tensor_add(out=exs[:, :w], in0=ex[:, 0, :w],
                                     in1=ex[:, 1, :w])
                pending.append((mi, m, off, w, ex, exs))
                if len(pending) > 2:
                    emit_pv(*pending.pop(0))
                # keep interleaved units on pace: after item mi, the number
                # of units left should be at most nu*(nit-1-mi)/nit
                while uq and len(uq) * nit > len(units) * (nit - 1 - mi):
                    uq.pop(0)()
            recip = misc.tile([P, 512], F32, tag="recip")
            if tail and variant == "base":
                while uq:
                    uq.pop(0)()
                for args in pending:
                    emit_pv(*args)
                # kernel tail: per-128-col normalize so each output-projection
                # block launches as soon as its columns are ready
                fin = fin_pool.tile([P, 4, d], F32, tag="fin")
                for blk in range(4):
                    lo = blk * P
                    nc.vector.reciprocal(out=recip[:, lo:lo + P],
                                         in_=den_ps[:, lo:lo + P])
                    nc.vector.tensor_mul(out=otn_all[:, j, lo:lo + P],
                                         in0=acc_ps[:, lo:lo + P],
                                         in1=recip[:, lo:lo + P])
                    outproj_block(j, blk, fin, tail=True)
                return
            for args in pending:
                emit_pv(*args)
            while uq:
                uq.pop(0)()
            # half-pipelined normalize shortens the DVE chain that gates the
            # next chunk's accumulator-slot reuse
            for lo in (0, 256):
                nc.vector.reciprocal(out=recip[:, lo:lo + 256],
                                     in_=den_ps[:, lo:lo + 256])
                nc.vector.tensor_mul(out=otn_all[:, j, lo:lo + 256],
                                     in0=acc_ps[:, lo:lo + 256],
                                     in1=recip[:, lo:lo + 256])

        # ---- top-level schedule ----
        if variant == "dmaonly":
            for s2 in range(NSC // 2):
                xt = emit_x_dma(s2)
                for blk in range(4):
                    vq = 4 * s2 + blk
                    fin = fin_pool.tile([P, d], F32, tag="dfin")
                    nc.vector.tensor_copy(out=fin[:], in_=xt[:, blk, :])
                    oeng = nc.sync if blk % 2 == 0 else nc.gpsimd
                    oeng.dma_start(out=outd[vq * P:(vq + 1) * P, :],
                                   in_=fin[:])
        elif variant == "noattn":
            for s2 in range(NSC // 2):
                xt = emit_x_dma(s2)
                for u in make_proj_units(s2, xt):
                    u()
                nc.vector.tensor_copy(out=otn_all[:, s2, :],
                                      in_=QT[:, s2 * 512:(s2 + 1) * 512])
                for u in make_outproj_units(s2, tail=(s2 == NSC // 2 - 1)):
                    u()
        else:
            xt = emit_x_dma(0)
            for u in make_proj_units(0, xt):
                u()
            for j in range(NQC):
                units = []
                if j > 0:
                    units += make_outproj_units(j - 1)
                if j + 1 < NQC:
                    xt = emit_x_dma(j + 1)
                    units += make_proj_units(j + 1, xt)
                emit_attention(j, units)
            if variant != "base":
                for u in make_outproj_units(NQC - 1, tail=True):
                    u()

    _split_excess_waits(nc)
    return nc


def make_masks(p, dtype=np.float32):
    """mask[e, t, 128u+r] = 1 iff virtual-kv (block e, offset t) is attended
    by virtual-q (block u, offset r) of the same 512-aligned q-chunk."""
    e = np.arange(8)[:, None, None]
    t = np.arange(P)[None, :, None]
    sq = np.arange(512)[None, None, :]
    u, r = sq // P, sq % P
    kv_pos = (e ^ p) * P + t
    q_pos = 256 * u + P * p + r
    return (kv_pos <= q_pos).astype(dtype)


def shard_inputs(x, Wq, bq, Wk, bk, Wv, bv, Wo, bo):
    """Build per-core input maps (and the host-side residual bias)."""
    import ml_dtypes

    bf16 = ml_dtypes.bfloat16
    x = np.asarray(x, dtype=np.float32)

    def packw(W):
        # [D, H] -> [P, DC*H] with row dc*128+p at [p, dc*H:(dc+1)*H]:
        # contiguous 2KB per partition line for descriptor-friendly DMA
        wt = np.asarray(W, np.float32).T.astype(bf16)          # [D, H]
        return np.ascontiguousarray(
            wt.reshape(D // P, P, H).transpose(1, 0, 2).reshape(P, -1))

    wq_t, wk_t, wv_t = packw(Wq), packw(Wk), packw(Wv)
    wo_t = np.asarray(Wo, np.float32).T.astype(bf16)  # [H, D]
    bqk = np.stack([np.asarray(bq, np.float32),
                    np.asarray(bk, np.float32)], axis=1).astype(bf16)
    # const = [wk | bqk | wv | wq | wo | ones | mask], one bf16 tensor
    consts = []
    for p in range(2):
        c = np.empty((P, 4 * D + 2 + P + 8 * 512), bf16)
        c[:, 0:D] = wk_t
        c[:, D:D + 2] = bqk
        c[:, D + 2:2 * D + 2] = wv_t
        c[:, 2 * D + 2:3 * D + 2] = wq_t
        c[:, 3 * D + 2:4 * D + 2] = wo_t
        c[:, 4 * D + 2:4 * D + 2 + P] = 1
        c[:, 4 * D + 2 + P:] = np.ascontiguousarray(
            make_masks(p, bf16).transpose(1, 0, 2)).reshape(P, 8 * 512)
        consts.append(c)
    # bv and bo are applied on the host: softmax rows sum to one, so
    # attn@(V+bv) @ Wo^T + bo = attn@V @ Wo^T + (Wo@bv + bo).
    bo_eff = (np.asarray(Wo, np.float32) @ np.asarray(bv, np.float32)
              + np.asarray(bo, np.float32))

    in_maps = []
    nblk = S // P
    for c in range(NCORES):
        b, p = c // 2, c % 2
        xb = x[b]
        if p:
            perm = np.arange(nblk) ^ 1
            xb = xb.reshape(nblk, P, D)[perm].reshape(S, D)
        xT = np.ascontiguousarray(xb.T.astype(bf16))
        in_maps.append({"xT": xT, "const": consts[p]})
    return in_maps, bo_eff


def gather_outputs(results, bo_eff):
    out = np.empty((B, S, D), np.float32)
    for c in range(NCORES):
        b, p = c // 2, c % 2
        co = results[c]["out"]           # [S//2, D]
        blocks = co.reshape(S // 2 // P, P, D)
        out[b, :, :].reshape(S // P, P, D)[2 * np.arange(S // 2 // P) + p] = blocks
    out += bo_eff[None, None, :]
    return out


_prog_cache = {}


def _get_program():
    if "nc" not in _prog_cache:
        _prog_cache["nc"] = build_program()
    return _prog_cache["nc"]


def kernel(x, Wq, bq, Wk, bk, Wv, bv, Wo, bo):
    from concourse.bass_utils import run_bass_kernel_spmd

    nc = _get_program()
    in_maps, bo_eff = shard_inputs(x, Wq, bq, Wk, bk, Wv, bv, Wo, bo)
    res = run_bass_kernel_spmd(nc, in_maps, core_ids=list(range(NCORES)))
    return gather_outputs(res.results, bo_eff)
